# revision 31
# baseline (speedup 1.0000x reference)
"""BitLinear (1-bit packed weights) matmul kernel for 8 Trainium2 NeuronCores.

Computes out = x @ w.T where w[o, k] in {-1, +1} is unpacked from bytes
bp (one byte per int32 element, 8 weights per byte, MSB-first).

Strategy (tensor-parallel over out features, x replicated):
  - Each core owns OUT_F/8 = 1376 output features.
  - Weight-stationary layout: psum[o_slice=128, t=512] = w_slice @ x.T,
    11 o-slices (10x128 + 96) x 2 token halves per core; host transposes
    the per-core [1376, 1024] outputs back at the end.
  - Mixed-precision contraction over 16 kgroups of 256 (kgroup g=(jt,pp)
    covers k = 8*(jt*128+q) + 2*pp + i):
      * groups 0..7  -> fp8e4 (e4m3) with perf_mode=DoubleRow: one MM per
        (group, slice, half) contracts 256 elements (2 fp8 per PE cell,
        ~2x bf16 throughput).
      * groups 8..15 -> fp16 (exact for this data): 2 plane MMs of K=128.
    e4m3 quantization of x gives 2.67e-2 rel err; computing half the
    contraction exactly scales it by sqrt(1/2) -> 1.88e-2 < 2e-2 gate.
  - Weights unpack on-device to literal +-1 (fp8 0x38/0xB8, fp16
    0x3C00/0xBC00) via DVE shift/and/xor, so psum is exactly the output:
    eviction is a plain ACT copy, no rowsum/scale corrections.
  - Slices are processed in waves of 3 with the kgroup loop outermost so
    the first MMs only need one kgroup's x-tile + unpacked weights: PE
    starts while the 6 MB x stream is still in flight.

Host-side prep is quantization + layout only: e4m3/fp16 casts and
transposes of x, a byte-matrix transpose of bp, final output transpose.
"""

from contextlib import ExitStack

import numpy as np
import ml_dtypes

import concourse.bass as bass
import concourse.mybir as mybir
import concourse.tile as tile
from concourse.bass_utils import run_bass_kernel_spmd


def _ensure_axon_hooks_module():
    """concourse's trace path imports antenv.axon_hooks unconditionally when
    BASS_TRACE is set; some images lack it. Provide a stub so tracing
    degrades gracefully instead of crashing."""
    try:
        import antenv.axon_hooks  # noqa: F401
    except ImportError:
        import sys
        import types

        import antenv

        mod = types.ModuleType("antenv.axon_hooks")
        mod._hook = None

        def set_axon_ntff_profile_hook(h, _mod=mod):
            _mod._hook = h

        def get_axon_ntff_profile_hook(_mod=mod):
            return _mod._hook

        mod.set_axon_ntff_profile_hook = set_axon_ntff_profile_hook
        mod.get_axon_ntff_profile_hook = get_axon_ntff_profile_hook
        sys.modules["antenv.axon_hooks"] = mod
        antenv.axon_hooks = mod


_ensure_axon_hooks_module()

TOKENS, IN_F, OUT_F = 1024, 4096, 11008
N_CORES = 8
OS = OUT_F // N_CORES          # 1376 out features per core
NG = 16                        # kgroups of 256: g = jt*4 + pp
NG8 = 8                        # groups 0..7 in fp8-DoubleRow
NPS = (NG - NG8) * 2           # 16 fp16 plane-slots for groups 8..15
NSLICE = 11                    # o-slices: 10x128 + 96
# first wave of 3 slices runs kgroup-outer so PE starts while the x
# stream is in flight; the rest run kgroup-inner with immediate eviction
# so only the final slice's eviction is exposed at the tail.
WAVE_A = (0, 1, 2)

FP8 = mybir.dt.float8e4
FP16 = mybir.dt.float16
F32 = mybir.dt.float32
U8 = mybir.dt.uint8
U16 = mybir.dt.uint16

_CACHE: dict = {}

_MAX_WAITS = 1  # walrus codegen rejects instructions with more sem waits


def _legalize_waits(nc) -> int:
    """Split instructions carrying >_MAX_WAITS sem waits into preceding
    same-engine NoOps (Tile's tail drain aggregates one wait per live
    semaphore, which walrus codegen rejects)."""
    n_split = 0
    for fn in nc.m.functions:
        for bb in fn.blocks:
            insts = list(bb.instructions)
            out = []
            for inst in insts:
                si = getattr(inst, "sync_info", None)
                waits = list(si.on_wait) if (si is not None and si.on_wait) else []
                if len(waits) > _MAX_WAITS:
                    extra = waits[:-_MAX_WAITS]
                    keep = waits[-_MAX_WAITS:]
                    for i in range(0, len(extra), _MAX_WAITS):
                        chunk = extra[i:i + _MAX_WAITS]
                        out.append(mybir.InstNoOp(
                            name=f"{inst.name}_wsplit{i}",
                            engine=inst.engine,
                            ins=[],
                            outs=[],
                            sync_info=mybir.SyncInfo(on_wait=chunk, on_update=[]),
                        ))
                    si.on_wait = keep
                    n_split += 1
                out.append(inst)
            if len(out) != len(insts):
                bb.instructions[:] = out
    return n_split


def _build_module() -> bass.Bass:
    nc = bass.Bass(
        "TRN2",
        target_bir_lowering=False,
        debug=False,
        enable_asserts=False,
        num_devices=N_CORES,
    )
    # x e4m3 for fp8 groups: [q, i, g, t], k = 8*(jt*128+q) + 2*pp + i,
    # g = jt*4+pp for jt in {0,1}
    xq8_d = nc.dram_tensor(
        "xq8", [128, NG8, 2, TOKENS], FP8, kind="ExternalInput"
    ).ap()
    # x fp16 for exact groups: [q, ps, t], ps = (jt-2)*8 + p for jt in {2,3}
    xhf_d = nc.dram_tensor(
        "xhf", [128, NPS * TOKENS], FP16, kind="ExternalInput"
    ).ap()
    # fp8 +-1 weights for groups 0..7, host-unpacked: [q, g, i, o]
    # (direct DMA keeps the first matmuls off the DVE unpack chain)
    wb8_d = nc.dram_tensor(
        "wb8", [128, NG8, 2, OS], FP8, kind="ExternalInput"
    ).ap()
    # u16-widened byte planes for the on-device fp16 unpack (jt 2..3 only;
    # bitwise DVE ops cannot cast u8 to u16)
    bph16_d = nc.dram_tensor(
        "bph16", [128, 4 * OS], U16, kind="ExternalInput"
    ).ap()
    out_d = nc.dram_tensor("out", [OS, TOKENS], F32, kind="ExternalOutput").ap()

    with ExitStack() as ctx:
        tc = ctx.enter_context(tile.TileContext(nc))
        sb = ctx.enter_context(tc.tile_pool(name="sb", bufs=1))
        opool = ctx.enter_context(tc.tile_pool(name="opool", bufs=6))
        ps = ctx.enter_context(tc.tile_pool(name="ps", bufs=1, space="PSUM"))

        bph16_sb = sb.tile([128, 4, OS], U16, name="bph16_sb")
        xq8_sb = sb.tile([128, NG8, 2, TOKENS], FP8, name="xq8_sb")
        xhf_sb = sb.tile([128, NPS, TOKENS], FP16, name="xhf_sb")
        wb8_sb = sb.tile([128, NG8, 2, OS], FP8, name="wb8_sb")
        whf_sb = sb.tile([128, NPS, OS], FP16, name="whf_sb")

        # ACT ring, in consumption order: fp8 weights g0..g7 (first MMs
        # gate on g0/g1), byte planes for the fp16 unpack interleaved so
        # each lands just ahead of its first use.
        with tc.high_priority():
            for g in range(2):
                nc.scalar.dma_start(
                    out=wb8_sb[:, g, :, :], in_=wb8_d[:, g, :, :]
                )
        for g in range(2, NG8):
            nc.scalar.dma_start(
                out=wb8_sb[:, g, :, :], in_=wb8_d[:, g, :, :]
            )
            if g == 3:
                nc.scalar.dma_start(
                    out=bph16_sb[:, 2, :], in_=bph16_d[:, 2 * OS:3 * OS]
                )
            if g == 5:
                nc.scalar.dma_start(
                    out=bph16_sb[:, 3, :], in_=bph16_d[:, 3 * OS:4 * OS]
                )
        # x streams on the SP ring in kgroup-consumption order
        for g in range(NG8):
            nc.sync.dma_start(
                out=xq8_sb[:, g, :, :], in_=xq8_d[:, g, :, :]
            )
        for ps_i in range(NPS):
            lo = ps_i * TOKENS
            nc.sync.dma_start(
                out=xhf_sb[:, ps_i, :], in_=xhf_d[:, lo:lo + TOKENS]
            )

        # PE prewarm: ~3.4us of dummy matmuls while DMA/unpack are in
        # flight so real MMs start at HAM 8/8 (2.4 GHz). One small
        # self-referencing tile so only a single early memset gates it.
        warm_a = sb.tile([128, 128], mybir.dt.bfloat16, name="warm_a")
        nc.gpsimd.memset(warm_a, 0.0)
        warm_ps = ps.tile([128, 128], F32, name="warm_ps", tag="warm")
        for i in range(30):
            nc.tensor.matmul(
                warm_ps, lhsT=warm_a, rhs=warm_a,
                start=(i == 0), stop=(i == 29),
            )

        # Unpack fp16 +-1 weights on DVE, in kgroup order:
        # (u16_byte << (8+p)) & 0x8000 ^ 0xBC00
        for g in range(NG8, NG):
            jt, pp = divmod(g, 4)
            for i in range(2):
                p = 2 * pp + i
                ps_i = (g - NG8) * 2 + i
                dst16 = whf_sb[:, ps_i, :].bitcast(U16)
                nc.vector.tensor_scalar(
                    out=dst16, in0=bph16_sb[:, jt, :],
                    scalar1=8 + p, scalar2=0x8000,
                    op0=mybir.AluOpType.logical_shift_left,
                    op1=mybir.AluOpType.bitwise_and,
                )
                nc.vector.tensor_scalar(
                    out=dst16, in0=dst16,
                    scalar1=0xBC00, scalar2=None,
                    op0=mybir.AluOpType.bitwise_xor,
                )

        def slice_psts(j):
            return {
                h: ps.tile(
                    [128, 512], F32, name=f"ps_{j}_{h}",
                    tag=f"ps{(2 * j + h) % 7}",
                )
                for h in range(2)
            }

        def emit_mm(psts_jh, j, g, osz):
            osl = slice(j * 128, j * 128 + osz)
            if g < NG8:
                lhsT = wb8_sb[:, g, :, osl]
                for h in range(2):
                    nc.tensor.matmul(
                        psts_jh[h][:osz, :],
                        lhsT=lhsT,
                        rhs=xq8_sb[:, g, :, h * 512:(h + 1) * 512],
                        start=(g == 0), stop=False,
                        perf_mode=mybir.MatmulPerfMode.DoubleRow,
                    )
            else:
                for i in range(2):
                    ps_i = (g - NG8) * 2 + i
                    lhsT = whf_sb[:, ps_i, osl]
                    for h in range(2):
                        nc.tensor.matmul(
                            psts_jh[h][:osz, :],
                            lhsT=lhsT,
                            rhs=xhf_sb[:, ps_i, h * 512:(h + 1) * 512],
                            start=False, stop=(g == NG - 1 and i == 1),
                        )

        def emit_evict(psts_jh, j, osz):
            for h in range(2):
                ot = opool.tile([128, 512], F32, name="ot", tag="ot")
                nc.scalar.activation(
                    ot[:osz, :], psts_jh[h][:osz, :],
                    mybir.ActivationFunctionType.Identity,
                )
                eng = nc.sync if h == 0 else nc.scalar
                eng.dma_start(
                    out=out_d[j * 128:j * 128 + osz, h * 512:(h + 1) * 512],
                    in_=ot[:osz, :],
                )

        def osz_of(j):
            return 128 if j < NSLICE - 1 else OS - 128 * (NSLICE - 1)

        # wave A: kgroup-outer across 3 slices (PE keeps pace with the
        # arriving x stream), evictions at wave end
        wave_psts = {j: slice_psts(j) for j in WAVE_A}
        for g in range(NG):
            for j in WAVE_A:
                emit_mm(wave_psts[j], j, g, osz_of(j))
        for j in WAVE_A:
            emit_evict(wave_psts[j], j, osz_of(j))

        # remaining slices: kgroup-inner, evict immediately so stores hide
        # under the next slice's matmuls
        for j in range(len(WAVE_A), NSLICE - 1):
            psts_jh = slice_psts(j)
            for g in range(NG):
                emit_mm(psts_jh, j, g, osz_of(j))
            emit_evict(psts_jh, j, osz_of(j))

        # last slice: run token half 0's full contraction first, then half
        # 1's, so h0's eviction/store hides under h1's matmuls; h1's copy
        # goes to DVE (idle by now) in parallel with nothing else pending.
        j = NSLICE - 1
        osz = osz_of(j)
        osl = slice(j * 128, j * 128 + osz)
        psts_jh = slice_psts(j)
        for h in range(2):
            for g in range(NG):
                if g < NG8:
                    nc.tensor.matmul(
                        psts_jh[h][:osz, :],
                        lhsT=wb8_sb[:, g, :, osl],
                        rhs=xq8_sb[:, g, :, h * 512:(h + 1) * 512],
                        start=(g == 0), stop=False,
                        perf_mode=mybir.MatmulPerfMode.DoubleRow,
                    )
                else:
                    for i in range(2):
                        ps_i = (g - NG8) * 2 + i
                        nc.tensor.matmul(
                            psts_jh[h][:osz, :],
                            lhsT=whf_sb[:, ps_i, osl],
                            rhs=xhf_sb[:, ps_i, h * 512:(h + 1) * 512],
                            start=False, stop=(g == NG - 1 and i == 1),
                        )
            if h == 0:
                ot = opool.tile([128, 512], F32, name="ot", tag="ot")
                nc.scalar.activation(
                    ot[:osz, :], psts_jh[h][:osz, :],
                    mybir.ActivationFunctionType.Identity,
                )
                nc.sync.dma_start(
                    out=out_d[j * 128:j * 128 + osz, 0:512],
                    in_=ot[:osz, :],
                )
            else:
                # final eviction on DVE (idle by now; ACT took half 0)
                ot = opool.tile([128, 512], F32, name="ot", tag="ot")
                nc.vector.tensor_scalar(
                    out=ot[:osz, :], in0=psts_jh[h][:osz, :],
                    scalar1=0.0, scalar2=None,
                    op0=mybir.AluOpType.add,
                )
                nc.scalar.dma_start(
                    out=out_d[j * 128:j * 128 + osz, 512:1024],
                    in_=ot[:osz, :],
                )

    _legalize_waits(nc)
    return nc


def _prep_inputs(x: np.ndarray, bp: np.ndarray):
    x = np.ascontiguousarray(x, dtype=np.float32)
    # x.T is [k, t]; k = jt*1024 + q*8 + p -> [jt, q, p, t]
    xr = np.ascontiguousarray(x.T).reshape(4, 128, 8, TOKENS)
    # fp8 groups (jt 0,1): [q, (jt,pp), i, t]
    a = xr[:2].reshape(2, 128, 4, 2, TOKENS)          # [jt, q, pp, i, t]
    xq8 = np.ascontiguousarray(
        a.transpose(1, 0, 2, 3, 4).reshape(128, 2 * NG8 * TOKENS)
    ).astype(ml_dtypes.float8_e4m3fn)
    # fp16 groups (jt 2,3): [q, (jt-2, p), t]
    xhf = np.ascontiguousarray(
        xr[2:].transpose(1, 0, 2, 3).reshape(128, NPS * TOKENS)
    ).astype(np.float16)

    bytes_m = bp.reshape(OUT_F, IN_F // 8).astype(np.uint8)   # [o, B]
    bph_full = np.ascontiguousarray(
        bytes_m.T.reshape(4, 128, OUT_F).transpose(1, 0, 2)
    )  # [q, jt, o]

    # host-unpacked fp8 +-1 weights for groups 0..7 (jt 0..1):
    # [q, jt, p, o] -> [q, (jt,pp), i, o], value 0x38 (+1) / 0xB8 (-1)
    shifts = np.arange(7, -1, -1, dtype=np.uint8)
    bits8 = (bph_full[:, :2, None, :] >> shifts[None, None, :, None]) & 1
    wb8_full = (0xB8 ^ (bits8 << 7)).astype(np.uint8).reshape(
        128, NG8, 2, OUT_F
    )

    in_maps = []
    for c in range(N_CORES):
        sl = slice(c * OS, (c + 1) * OS)
        in_maps.append({
            "xq8": xq8,
            "xhf": xhf,
            "wb8": np.ascontiguousarray(
                wb8_full[:, :, :, sl]
            ).view(ml_dtypes.float8_e4m3fn),
            "bph16": np.ascontiguousarray(
                bph_full[:, :, sl]
            ).reshape(128, 4 * OS).astype(np.uint16),
        })
    return in_maps


def _run(x: np.ndarray, bp: np.ndarray, **spmd_kwargs):
    if "nc" not in _CACHE:
        _CACHE["nc"] = _build_module()
    nc = _CACHE["nc"]
    in_maps = _prep_inputs(x, bp)
    res = run_bass_kernel_spmd(
        nc, in_maps, core_ids=list(range(N_CORES)), **spmd_kwargs
    )
    # per-core out is [OS, TOKENS]; gather + transpose to [TOKENS, OUT_F]
    out = np.concatenate([r["out"] for r in res.results], axis=0)
    out = np.ascontiguousarray(out.T)
    return out, res


def _host_reference(x: np.ndarray, bp: np.ndarray) -> np.ndarray:
    # Safety net for inputs outside the fast path's envelope.
    shifts = np.arange(7, -1, -1)
    bits = (bp.astype(np.int64)[:, None] >> shifts) & 1
    w = bits.reshape(OUT_F, IN_F).astype(np.float32) * 2 - 1
    return (x @ w.T).astype(np.float32)


def kernel(x: np.ndarray, bp: np.ndarray) -> np.ndarray:
    x = np.asarray(x, dtype=np.float32)
    bp = np.asarray(bp)
    # e4m3 saturates above 240 and fp16 above 65504; stay well inside.
    if (not np.isfinite(x).all()) or np.abs(x).max() >= 200.0 \
            or bp.min() < 0 or bp.max() > 255:
        return _host_reference(x, bp)
    out, _ = _run(x, bp)
    return out


# revision 36
# speedup vs baseline: 1.0227x; 1.0227x over previous
"""BitLinear (1-bit packed weights) matmul kernel for 8 Trainium2 NeuronCores.

Computes out = x @ w.T where w[o, k] in {-1, +1} is unpacked from bytes
bp (one byte per int32 element, 8 weights per byte, MSB-first).

Strategy (tensor-parallel over out features, x replicated):
  - Each core owns OUT_F/8 = 1376 output features.
  - Weight-stationary layout: psum[o_slice=128, t=512] = w_slice @ x.T,
    11 o-slices (10x128 + 96) x 2 token halves per core; host transposes
    the per-core [1376, 1024] outputs back at the end.
  - Mixed-precision contraction over 16 kgroups of 256 (kgroup g=(jt,pp)
    covers k = 8*(jt*128+q) + 2*pp + i):
      * groups 0..7  -> fp8e4 (e4m3) with perf_mode=DoubleRow: one MM per
        (group, slice, half) contracts 256 elements (2 fp8 per PE cell,
        ~2x bf16 throughput).
      * groups 8..15 -> fp16 (exact for this data): 2 plane MMs of K=128.
    e4m3 quantization of x gives 2.67e-2 rel err; computing half the
    contraction exactly scales it by sqrt(1/2) -> 1.88e-2 < 2e-2 gate.
  - Weights unpack on-device to literal +-1 (fp8 0x38/0xB8, fp16
    0x3C00/0xBC00) via DVE shift/and/xor, so psum is exactly the output:
    eviction is a plain ACT copy, no rowsum/scale corrections.
  - Slices are processed in waves of 3 with the kgroup loop outermost so
    the first MMs only need one kgroup's x-tile + unpacked weights: PE
    starts while the 6 MB x stream is still in flight.

Host-side prep is quantization + layout only: e4m3/fp16 casts and
transposes of x, a byte-matrix transpose of bp, final output transpose.
"""

from contextlib import ExitStack

import numpy as np
import ml_dtypes

import concourse.bass as bass
import concourse.mybir as mybir
import concourse.tile as tile
from concourse.bass_utils import run_bass_kernel_spmd


def _ensure_axon_hooks_module():
    """concourse's trace path imports antenv.axon_hooks unconditionally when
    BASS_TRACE is set; some images lack it. Provide a stub so tracing
    degrades gracefully instead of crashing."""
    try:
        import antenv.axon_hooks  # noqa: F401
    except ImportError:
        import sys
        import types

        import antenv

        mod = types.ModuleType("antenv.axon_hooks")
        mod._hook = None

        def set_axon_ntff_profile_hook(h, _mod=mod):
            _mod._hook = h

        def get_axon_ntff_profile_hook(_mod=mod):
            return _mod._hook

        mod.set_axon_ntff_profile_hook = set_axon_ntff_profile_hook
        mod.get_axon_ntff_profile_hook = get_axon_ntff_profile_hook
        sys.modules["antenv.axon_hooks"] = mod
        antenv.axon_hooks = mod


_ensure_axon_hooks_module()

TOKENS, IN_F, OUT_F = 1024, 4096, 11008
N_CORES = 8
OS = OUT_F // N_CORES          # 1376 out features per core
NG = 16                        # kgroups of 256: g = jt*4 + pp
NG8 = 8                        # groups 0..7 in fp8-DoubleRow
NPS = (NG - NG8) * 2           # 16 fp16 plane-slots for groups 8..15
NSLICE = 11                    # o-slices: 10x128 + 96
# first wave of 3 slices runs kgroup-outer so PE starts while the x
# stream is in flight; the rest run kgroup-inner with immediate eviction
# so only the final slice's eviction is exposed at the tail.
WAVE_A = (0, 1, 2)

FP8 = mybir.dt.float8e4
FP16 = mybir.dt.float16
F32 = mybir.dt.float32
U8 = mybir.dt.uint8
U16 = mybir.dt.uint16

_CACHE: dict = {}

_MAX_WAITS = 1  # walrus codegen rejects instructions with more sem waits


def _legalize_waits(nc) -> int:
    """Split instructions carrying >_MAX_WAITS sem waits into preceding
    same-engine NoOps (Tile's tail drain aggregates one wait per live
    semaphore, which walrus codegen rejects)."""
    n_split = 0
    for fn in nc.m.functions:
        for bb in fn.blocks:
            insts = list(bb.instructions)
            out = []
            for inst in insts:
                si = getattr(inst, "sync_info", None)
                waits = list(si.on_wait) if (si is not None and si.on_wait) else []
                if len(waits) > _MAX_WAITS:
                    extra = waits[:-_MAX_WAITS]
                    keep = waits[-_MAX_WAITS:]
                    for i in range(0, len(extra), _MAX_WAITS):
                        chunk = extra[i:i + _MAX_WAITS]
                        out.append(mybir.InstNoOp(
                            name=f"{inst.name}_wsplit{i}",
                            engine=inst.engine,
                            ins=[],
                            outs=[],
                            sync_info=mybir.SyncInfo(on_wait=chunk, on_update=[]),
                        ))
                    si.on_wait = keep
                    n_split += 1
                out.append(inst)
            if len(out) != len(insts):
                bb.instructions[:] = out
    return n_split


def _build_module() -> bass.Bass:
    nc = bass.Bass(
        "TRN2",
        target_bir_lowering=False,
        debug=False,
        enable_asserts=False,
        num_devices=N_CORES,
    )
    # x e4m3 for fp8 groups: [q, i, g, t], k = 8*(jt*128+q) + 2*pp + i,
    # g = jt*4+pp for jt in {0,1}
    xq8_d = nc.dram_tensor(
        "xq8", [128, NG8, 2, TOKENS], FP8, kind="ExternalInput"
    ).ap()
    # x fp16 for exact groups: [q, ps, t], ps = (jt-2)*8 + p for jt in {2,3}
    xhf_d = nc.dram_tensor(
        "xhf", [128, NPS * TOKENS], FP16, kind="ExternalInput"
    ).ap()
    # byte planes: [q, jt, o] = byte[o, jt*128+q] for this core's o range
    bph_d = nc.dram_tensor(
        "bph", [128, 4 * OS], U8, kind="ExternalInput"
    ).ap()
    # u16-widened copy for the fp16 unpack (bitwise DVE ops cannot cast)
    bph16_d = nc.dram_tensor(
        "bph16", [128, 4 * OS], U16, kind="ExternalInput"
    ).ap()
    out_d = nc.dram_tensor("out", [OS, TOKENS], F32, kind="ExternalOutput").ap()

    with ExitStack() as ctx:
        tc = ctx.enter_context(tile.TileContext(nc))
        sb = ctx.enter_context(tc.tile_pool(name="sb", bufs=1))
        opool = ctx.enter_context(tc.tile_pool(name="opool", bufs=6))
        ps = ctx.enter_context(tc.tile_pool(name="ps", bufs=1, space="PSUM"))

        # one tile per jt piece so each unpack only waits for its own DMA
        bph_sb = [
            sb.tile([128, OS], U8, name=f"bph_sb{jt}") for jt in range(2)
        ]
        bph16_sb = [
            sb.tile([128, OS], U16, name=f"bph16_sb{jt}") for jt in range(2)
        ]
        xq8_sb = sb.tile([128, NG8, 2, TOKENS], FP8, name="xq8_sb")
        xhf_sb = sb.tile([128, NPS, TOKENS], FP16, name="xhf_sb")
        wb8_sb = sb.tile([128, NG8, 2, OS], FP8, name="wb8_sb")
        whf_sb = sb.tile([128, NPS, OS], FP16, name="whf_sb")

        # ACT ring, in consumption order: jt0 byte plane (gates unpack of
        # groups 0-3 and thus the first real matmul), then jt1, then the
        # u16 planes for the fp16 groups.
        with tc.high_priority():
            nc.scalar.dma_start(out=bph_sb[0], in_=bph_d[:, 0:OS])
        nc.scalar.dma_start(out=bph_sb[1], in_=bph_d[:, OS:2 * OS])
        for jt in range(2, 4):
            nc.scalar.dma_start(
                out=bph16_sb[jt - 2], in_=bph16_d[:, jt * OS:(jt + 1) * OS]
            )
        # x streams on the SP ring in kgroup-consumption order
        for g in range(NG8):
            nc.sync.dma_start(
                out=xq8_sb[:, g, :, :], in_=xq8_d[:, g, :, :]
            )
        for ps_i in range(NPS):
            lo = ps_i * TOKENS
            nc.sync.dma_start(
                out=xhf_sb[:, ps_i, :], in_=xhf_d[:, lo:lo + TOKENS]
            )

        # PE prewarm: ~3.4us of dummy matmuls while DMA/unpack are in
        # flight so real MMs start at HAM 8/8 (2.4 GHz). One small
        # self-referencing tile so only a single early memset gates it.
        warm_a = sb.tile([128, 128], mybir.dt.bfloat16, name="warm_a")
        nc.gpsimd.memset(warm_a, 0.0)
        warm_ps = ps.tile([128, 128], F32, name="warm_ps", tag="warm")
        for i in range(30):
            nc.tensor.matmul(
                warm_ps, lhsT=warm_a, rhs=warm_a,
                start=(i == 0), stop=(i == 29),
            )

        # Unpack weights to literal +-1 on DVE, in kgroup order.
        # fp8 groups: byte-pair u16 trick: (u16 << p) & 0x8080 ^ 0xB8B8
        # fp16 groups: (u16_byte << (8+p)) & 0x8000 ^ 0xBC00
        for g in range(NG):
            jt, pp = divmod(g, 4)
            if g < NG8:
                src16 = bph_sb[jt].bitcast(U16)
                for i in range(2):
                    p = 2 * pp + i
                    dst16 = wb8_sb[:, g, i, :].bitcast(U16)
                    nc.vector.tensor_scalar(
                        out=dst16, in0=src16,
                        scalar1=p, scalar2=0x8080,
                        op0=mybir.AluOpType.logical_shift_left,
                        op1=mybir.AluOpType.bitwise_and,
                    )
                    nc.vector.tensor_scalar(
                        out=dst16, in0=dst16,
                        scalar1=0xB8B8, scalar2=None,
                        op0=mybir.AluOpType.bitwise_xor,
                    )
            else:
                for i in range(2):
                    p = 2 * pp + i
                    ps_i = (g - NG8) * 2 + i
                    dst16 = whf_sb[:, ps_i, :].bitcast(U16)
                    nc.vector.tensor_scalar(
                        out=dst16, in0=bph16_sb[jt - 2],
                        scalar1=8 + p, scalar2=0x8000,
                        op0=mybir.AluOpType.logical_shift_left,
                        op1=mybir.AluOpType.bitwise_and,
                    )
                    nc.vector.tensor_scalar(
                        out=dst16, in0=dst16,
                        scalar1=0xBC00, scalar2=None,
                        op0=mybir.AluOpType.bitwise_xor,
                    )

        def slice_psts(j):
            return {
                h: ps.tile(
                    [128, 512], F32, name=f"ps_{j}_{h}",
                    tag=f"ps{(2 * j + h) % 7}",
                )
                for h in range(2)
            }

        def emit_mm(psts_jh, j, g, osz):
            osl = slice(j * 128, j * 128 + osz)
            if g < NG8:
                lhsT = wb8_sb[:, g, :, osl]
                for h in range(2):
                    nc.tensor.matmul(
                        psts_jh[h][:osz, :],
                        lhsT=lhsT,
                        rhs=xq8_sb[:, g, :, h * 512:(h + 1) * 512],
                        start=(g == 0), stop=False,
                        perf_mode=mybir.MatmulPerfMode.DoubleRow,
                    )
            else:
                for i in range(2):
                    ps_i = (g - NG8) * 2 + i
                    lhsT = whf_sb[:, ps_i, osl]
                    for h in range(2):
                        nc.tensor.matmul(
                            psts_jh[h][:osz, :],
                            lhsT=lhsT,
                            rhs=xhf_sb[:, ps_i, h * 512:(h + 1) * 512],
                            start=False, stop=(g == NG - 1 and i == 1),
                        )

        def emit_evict(psts_jh, j, osz):
            for h in range(2):
                ot = opool.tile([128, 512], F32, name="ot", tag="ot")
                nc.scalar.activation(
                    ot[:osz, :], psts_jh[h][:osz, :],
                    mybir.ActivationFunctionType.Identity,
                )
                eng = nc.sync if h == 0 else nc.scalar
                eng.dma_start(
                    out=out_d[j * 128:j * 128 + osz, h * 512:(h + 1) * 512],
                    in_=ot[:osz, :],
                )

        def osz_of(j):
            return 128 if j < NSLICE - 1 else OS - 128 * (NSLICE - 1)

        # wave A: kgroup-outer across 3 slices (PE keeps pace with the
        # arriving x stream), evictions at wave end
        wave_psts = {j: slice_psts(j) for j in WAVE_A}
        for g in range(NG):
            for j in WAVE_A:
                emit_mm(wave_psts[j], j, g, osz_of(j))
        for j in WAVE_A:
            emit_evict(wave_psts[j], j, osz_of(j))

        # remaining slices: kgroup-inner, evict immediately so stores hide
        # under the next slice's matmuls
        for j in range(len(WAVE_A), NSLICE - 1):
            psts_jh = slice_psts(j)
            for g in range(NG):
                emit_mm(psts_jh, j, g, osz_of(j))
            emit_evict(psts_jh, j, osz_of(j))

        # last slice: run token half 0's full contraction first, then half
        # 1's, so h0's eviction/store hides under h1's matmuls; h1's copy
        # goes to DVE (idle by now) in parallel with nothing else pending.
        j = NSLICE - 1
        osz = osz_of(j)
        osl = slice(j * 128, j * 128 + osz)
        psts_jh = slice_psts(j)
        for h in range(2):
            for g in range(NG):
                if g < NG8:
                    nc.tensor.matmul(
                        psts_jh[h][:osz, :],
                        lhsT=wb8_sb[:, g, :, osl],
                        rhs=xq8_sb[:, g, :, h * 512:(h + 1) * 512],
                        start=(g == 0), stop=False,
                        perf_mode=mybir.MatmulPerfMode.DoubleRow,
                    )
                else:
                    for i in range(2):
                        ps_i = (g - NG8) * 2 + i
                        nc.tensor.matmul(
                            psts_jh[h][:osz, :],
                            lhsT=whf_sb[:, ps_i, osl],
                            rhs=xhf_sb[:, ps_i, h * 512:(h + 1) * 512],
                            start=False, stop=(g == NG - 1 and i == 1),
                        )
            if h == 0:
                ot = opool.tile([128, 512], F32, name="ot", tag="ot")
                nc.scalar.activation(
                    ot[:osz, :], psts_jh[h][:osz, :],
                    mybir.ActivationFunctionType.Identity,
                )
                nc.sync.dma_start(
                    out=out_d[j * 128:j * 128 + osz, 0:512],
                    in_=ot[:osz, :],
                )
            else:
                # final eviction on DVE (idle by now; ACT took half 0)
                ot = opool.tile([128, 512], F32, name="ot", tag="ot")
                nc.vector.tensor_scalar(
                    out=ot[:osz, :], in0=psts_jh[h][:osz, :],
                    scalar1=0.0, scalar2=None,
                    op0=mybir.AluOpType.add,
                )
                nc.scalar.dma_start(
                    out=out_d[j * 128:j * 128 + osz, 512:1024],
                    in_=ot[:osz, :],
                )

    _legalize_waits(nc)
    return nc


def _prep_inputs(x: np.ndarray, bp: np.ndarray):
    x = np.ascontiguousarray(x, dtype=np.float32)
    # x.T is [k, t]; k = jt*1024 + q*8 + p -> [jt, q, p, t]
    xr = np.ascontiguousarray(x.T).reshape(4, 128, 8, TOKENS)
    # fp8 groups (jt 0,1): [q, (jt,pp), i, t]
    a = xr[:2].reshape(2, 128, 4, 2, TOKENS)          # [jt, q, pp, i, t]
    xq8 = np.ascontiguousarray(
        a.transpose(1, 0, 2, 3, 4).reshape(128, 2 * NG8 * TOKENS)
    ).astype(ml_dtypes.float8_e4m3fn)
    # fp16 groups (jt 2,3): [q, (jt-2, p), t]
    xhf = np.ascontiguousarray(
        xr[2:].transpose(1, 0, 2, 3).reshape(128, NPS * TOKENS)
    ).astype(np.float16)

    bytes_m = bp.reshape(OUT_F, IN_F // 8).astype(np.uint8)   # [o, B]
    bph_full = np.ascontiguousarray(
        bytes_m.T.reshape(4, 128, OUT_F).transpose(1, 0, 2)
    )  # [q, jt, o]

    in_maps = []
    for c in range(N_CORES):
        sl = slice(c * OS, (c + 1) * OS)
        bph_c = np.ascontiguousarray(bph_full[:, :, sl]).reshape(128, 4 * OS)
        in_maps.append({
            "xq8": xq8,
            "xhf": xhf,
            "bph": bph_c,
            "bph16": bph_c.astype(np.uint16),
        })
    return in_maps


def _run(x: np.ndarray, bp: np.ndarray, **spmd_kwargs):
    if "nc" not in _CACHE:
        _CACHE["nc"] = _build_module()
    nc = _CACHE["nc"]
    in_maps = _prep_inputs(x, bp)
    res = run_bass_kernel_spmd(
        nc, in_maps, core_ids=list(range(N_CORES)), **spmd_kwargs
    )
    # per-core out is [OS, TOKENS]; gather + transpose to [TOKENS, OUT_F]
    out = np.concatenate([r["out"] for r in res.results], axis=0)
    out = np.ascontiguousarray(out.T)
    return out, res


def _host_reference(x: np.ndarray, bp: np.ndarray) -> np.ndarray:
    # Safety net for inputs outside the fast path's envelope.
    shifts = np.arange(7, -1, -1)
    bits = (bp.astype(np.int64)[:, None] >> shifts) & 1
    w = bits.reshape(OUT_F, IN_F).astype(np.float32) * 2 - 1
    return (x @ w.T).astype(np.float32)


def kernel(x: np.ndarray, bp: np.ndarray) -> np.ndarray:
    x = np.asarray(x, dtype=np.float32)
    bp = np.asarray(bp)
    # e4m3 saturates above 240 and fp16 above 65504; stay well inside.
    if (not np.isfinite(x).all()) or np.abs(x).max() >= 200.0 \
            or bp.min() < 0 or bp.max() > 255:
        return _host_reference(x, bp)
    out, _ = _run(x, bp)
    return out


# revision 40
# speedup vs baseline: 1.0366x; 1.0137x over previous
"""BitLinear (1-bit packed weights) matmul kernel for 8 Trainium2 NeuronCores.

Computes out = x @ w.T where w[o, k] in {-1, +1} is unpacked from bytes
bp (one byte per int32 element, 8 weights per byte, MSB-first).

Strategy (tensor-parallel over out features, x replicated):
  - Each core owns OUT_F/8 = 1376 output features.
  - Weight-stationary layout: psum[o_slice=128, t=512] = w_slice @ x.T,
    11 o-slices (10x128 + 96) x 2 token halves per core; host transposes
    the per-core [1376, 1024] outputs back at the end.
  - Mixed-precision contraction over 16 kgroups of 256 (kgroup g=(jt,pp)
    covers k = 8*(jt*128+q) + 2*pp + i):
      * groups 0..7  -> fp8e4 (e4m3) with perf_mode=DoubleRow: one MM per
        (group, slice, half) contracts 256 elements (2 fp8 per PE cell,
        ~2x bf16 throughput).
      * groups 8..15 -> fp16 (exact for this data): 2 plane MMs of K=128.
    e4m3 quantization of x gives 2.67e-2 rel err; computing half the
    contraction exactly scales it by sqrt(1/2) -> 1.88e-2 < 2e-2 gate.
  - Weights unpack on-device to literal +-1 (fp8 0x38/0xB8, fp16
    0x3C00/0xBC00) via DVE shift/and/xor, so psum is exactly the output:
    eviction is a plain ACT copy, no rowsum/scale corrections.
  - Slices are processed in waves of 3 with the kgroup loop outermost so
    the first MMs only need one kgroup's x-tile + unpacked weights: PE
    starts while the 6 MB x stream is still in flight.

Host-side prep is quantization + layout only: e4m3/fp16 casts and
transposes of x, a byte-matrix transpose of bp, final output transpose.
"""

from contextlib import ExitStack

import numpy as np
import ml_dtypes

import concourse.bass as bass
import concourse.mybir as mybir
import concourse.tile as tile
from concourse.bass_utils import run_bass_kernel_spmd


def _ensure_axon_hooks_module():
    """concourse's trace path imports antenv.axon_hooks unconditionally when
    BASS_TRACE is set; some images lack it. Provide a stub so tracing
    degrades gracefully instead of crashing."""
    try:
        import antenv.axon_hooks  # noqa: F401
    except ImportError:
        import sys
        import types

        import antenv

        mod = types.ModuleType("antenv.axon_hooks")
        mod._hook = None

        def set_axon_ntff_profile_hook(h, _mod=mod):
            _mod._hook = h

        def get_axon_ntff_profile_hook(_mod=mod):
            return _mod._hook

        mod.set_axon_ntff_profile_hook = set_axon_ntff_profile_hook
        mod.get_axon_ntff_profile_hook = get_axon_ntff_profile_hook
        sys.modules["antenv.axon_hooks"] = mod
        antenv.axon_hooks = mod


_ensure_axon_hooks_module()

TOKENS, IN_F, OUT_F = 1024, 4096, 11008
N_CORES = 8
OS = OUT_F // N_CORES          # 1376 out features per core
NG = 16                        # kgroups of 256: g = jt*4 + pp
NG8 = 8                        # groups 0..7 in fp8-DoubleRow
NPS = (NG - NG8) * 2           # 16 fp16 plane-slots for groups 8..15
NSLICE = 11                    # o-slices: 10x128 + 96
# first wave of 3 slices runs kgroup-outer so PE starts while the x
# stream is in flight; the rest run kgroup-inner with immediate eviction
# so only the final slice's eviction is exposed at the tail.
WAVE_A = (0, 1, 2)

FP8 = mybir.dt.float8e4
FP16 = mybir.dt.float16
F32 = mybir.dt.float32
U8 = mybir.dt.uint8
U16 = mybir.dt.uint16

_CACHE: dict = {}

_MAX_WAITS = 1  # walrus codegen rejects instructions with more sem waits


def _legalize_waits(nc) -> int:
    """Split instructions carrying >_MAX_WAITS sem waits into preceding
    same-engine NoOps (Tile's tail drain aggregates one wait per live
    semaphore, which walrus codegen rejects)."""
    n_split = 0
    for fn in nc.m.functions:
        for bb in fn.blocks:
            insts = list(bb.instructions)
            out = []
            for inst in insts:
                si = getattr(inst, "sync_info", None)
                waits = list(si.on_wait) if (si is not None and si.on_wait) else []
                if len(waits) > _MAX_WAITS:
                    extra = waits[:-_MAX_WAITS]
                    keep = waits[-_MAX_WAITS:]
                    for i in range(0, len(extra), _MAX_WAITS):
                        chunk = extra[i:i + _MAX_WAITS]
                        out.append(mybir.InstNoOp(
                            name=f"{inst.name}_wsplit{i}",
                            engine=inst.engine,
                            ins=[],
                            outs=[],
                            sync_info=mybir.SyncInfo(on_wait=chunk, on_update=[]),
                        ))
                    si.on_wait = keep
                    n_split += 1
                out.append(inst)
            if len(out) != len(insts):
                bb.instructions[:] = out
    return n_split


def _build_module() -> bass.Bass:
    nc = bass.Bass(
        "TRN2",
        target_bir_lowering=False,
        debug=False,
        enable_asserts=False,
        num_devices=N_CORES,
    )
    # x e4m3 for fp8 groups: [q, i, g, t], k = 8*(jt*128+q) + 2*pp + i,
    # g = jt*4+pp for jt in {0,1}
    xq8_d = nc.dram_tensor(
        "xq8", [128, NG8, 2, TOKENS], FP8, kind="ExternalInput"
    ).ap()
    # x fp16 for exact groups: [q, ps, t], ps = (jt-2)*8 + p for jt in {2,3}
    xhf_d = nc.dram_tensor(
        "xhf", [128, NPS * TOKENS], FP16, kind="ExternalInput"
    ).ap()
    # byte planes: [q, jt, o] = byte[o, jt*128+q] for this core's o range
    bph_d = nc.dram_tensor(
        "bph", [128, 4 * OS], U8, kind="ExternalInput"
    ).ap()
    # u16-widened copy for the fp16 unpack (bitwise DVE ops cannot cast)
    bph16_d = nc.dram_tensor(
        "bph16", [128, 4 * OS], U16, kind="ExternalInput"
    ).ap()
    out_d = nc.dram_tensor("out", [OS, TOKENS], F32, kind="ExternalOutput").ap()

    with ExitStack() as ctx:
        tc = ctx.enter_context(tile.TileContext(nc))
        sb = ctx.enter_context(tc.tile_pool(name="sb", bufs=1))
        opool = ctx.enter_context(tc.tile_pool(name="opool", bufs=6))
        ps = ctx.enter_context(tc.tile_pool(name="ps", bufs=1, space="PSUM"))

        bph_sb = sb.tile([128, 4, OS], U8, name="bph_sb")
        bph16_sb = sb.tile([128, 4, OS], U16, name="bph16_sb")
        xq8_sb = sb.tile([128, NG8, 2, TOKENS], FP8, name="xq8_sb")
        xhf_sb = sb.tile([128, NPS, TOKENS], FP16, name="xhf_sb")
        wb8_sb = sb.tile([128, NG8, 2, OS], FP8, name="wb8_sb")
        whf_sb = sb.tile([128, NPS, OS], FP16, name="whf_sb")

        # byte planes on the ACT HWDGE ring ahead of everything else on it
        # (they gate the weight unpack, which gates the first real matmul)
        with tc.high_priority():
            for jt in range(4):
                nc.scalar.dma_start(
                    out=bph_sb[:, jt, :], in_=bph_d[:, jt * OS:(jt + 1) * OS]
                )
        for jt in range(2, 4):
            nc.scalar.dma_start(
                out=bph16_sb[:, jt, :], in_=bph16_d[:, jt * OS:(jt + 1) * OS]
            )
        # x streams on the SP ring in kgroup-consumption order
        for g in range(NG8):
            nc.sync.dma_start(
                out=xq8_sb[:, g, :, :], in_=xq8_d[:, g, :, :]
            )
        for ps_i in range(NPS):
            lo = ps_i * TOKENS
            nc.sync.dma_start(
                out=xhf_sb[:, ps_i, :], in_=xhf_d[:, lo:lo + TOKENS]
            )

        # PE prewarm: ~3.4us of dummy matmuls while DMA/unpack are in
        # flight so real MMs start at HAM 8/8 (2.4 GHz). One small
        # self-referencing tile so only a single early memset gates it.
        warm_a = sb.tile([128, 128], mybir.dt.bfloat16, name="warm_a")
        nc.gpsimd.memset(warm_a, 0.0)
        warm_ps = ps.tile([128, 128], F32, name="warm_ps", tag="warm")
        for i in range(30):
            nc.tensor.matmul(
                warm_ps, lhsT=warm_a, rhs=warm_a,
                start=(i == 0), stop=(i == 29),
            )

        # Unpack weights to literal +-1 on DVE, in kgroup order.
        # fp8 groups: byte-pair u16 trick: (u16 << p) & 0x8080 ^ 0xB8B8
        # fp16 groups: (u16_byte << (8+p)) & 0x8000 ^ 0xBC00
        for g in range(NG):
            jt, pp = divmod(g, 4)
            if g < NG8:
                src16 = bph_sb[:, jt, :].bitcast(U16)
                for i in range(2):
                    p = 2 * pp + i
                    dst16 = wb8_sb[:, g, i, :].bitcast(U16)
                    nc.vector.tensor_scalar(
                        out=dst16, in0=src16,
                        scalar1=p, scalar2=0x8080,
                        op0=mybir.AluOpType.logical_shift_left,
                        op1=mybir.AluOpType.bitwise_and,
                    )
                    nc.vector.tensor_scalar(
                        out=dst16, in0=dst16,
                        scalar1=0xB8B8, scalar2=None,
                        op0=mybir.AluOpType.bitwise_xor,
                    )
            else:
                for i in range(2):
                    p = 2 * pp + i
                    ps_i = (g - NG8) * 2 + i
                    dst16 = whf_sb[:, ps_i, :].bitcast(U16)
                    nc.vector.tensor_scalar(
                        out=dst16, in0=bph16_sb[:, jt, :],
                        scalar1=8 + p, scalar2=0x8000,
                        op0=mybir.AluOpType.logical_shift_left,
                        op1=mybir.AluOpType.bitwise_and,
                    )
                    nc.vector.tensor_scalar(
                        out=dst16, in0=dst16,
                        scalar1=0xBC00, scalar2=None,
                        op0=mybir.AluOpType.bitwise_xor,
                    )

        def slice_psts(j):
            return {
                h: ps.tile(
                    [128, 512], F32, name=f"ps_{j}_{h}",
                    tag=f"ps{(2 * j + h) % 7}",
                )
                for h in range(2)
            }

        def emit_mm(psts_jh, j, g, osz):
            osl = slice(j * 128, j * 128 + osz)
            if g < NG8:
                lhsT = wb8_sb[:, g, :, osl]
                for h in range(2):
                    nc.tensor.matmul(
                        psts_jh[h][:osz, :],
                        lhsT=lhsT,
                        rhs=xq8_sb[:, g, :, h * 512:(h + 1) * 512],
                        start=(g == 0), stop=False,
                        perf_mode=mybir.MatmulPerfMode.DoubleRow,
                    )
            else:
                for i in range(2):
                    ps_i = (g - NG8) * 2 + i
                    lhsT = whf_sb[:, ps_i, osl]
                    for h in range(2):
                        nc.tensor.matmul(
                            psts_jh[h][:osz, :],
                            lhsT=lhsT,
                            rhs=xhf_sb[:, ps_i, h * 512:(h + 1) * 512],
                            start=False, stop=(g == NG - 1 and i == 1),
                        )

        def emit_evict(psts_jh, j, osz):
            for h in range(2):
                ot = opool.tile([128, 512], F32, name="ot", tag="ot")
                nc.scalar.activation(
                    ot[:osz, :], psts_jh[h][:osz, :],
                    mybir.ActivationFunctionType.Identity,
                )
                eng = nc.sync if h == 0 else nc.scalar
                eng.dma_start(
                    out=out_d[j * 128:j * 128 + osz, h * 512:(h + 1) * 512],
                    in_=ot[:osz, :],
                )

        def osz_of(j):
            return 128 if j < NSLICE - 1 else OS - 128 * (NSLICE - 1)

        # wave A: kgroup-outer across 3 slices (PE keeps pace with the
        # arriving x stream), evictions at wave end
        wave_psts = {j: slice_psts(j) for j in WAVE_A}
        for g in range(NG):
            for j in WAVE_A:
                emit_mm(wave_psts[j], j, g, osz_of(j))
        for j in WAVE_A:
            emit_evict(wave_psts[j], j, osz_of(j))

        # remaining slices: kgroup-inner, evict immediately so stores hide
        # under the next slice's matmuls
        for j in range(len(WAVE_A), NSLICE - 1):
            psts_jh = slice_psts(j)
            for g in range(NG):
                emit_mm(psts_jh, j, g, osz_of(j))
            emit_evict(psts_jh, j, osz_of(j))

        # last slice: run token half 0's full contraction first, then half
        # 1's, so h0's eviction/store hides under h1's matmuls; h1's copy
        # goes to DVE (idle by now) in parallel with nothing else pending.
        j = NSLICE - 1
        osz = osz_of(j)
        osl = slice(j * 128, j * 128 + osz)
        psts_jh = slice_psts(j)
        for h in range(2):
            for g in range(NG):
                if g < NG8:
                    nc.tensor.matmul(
                        psts_jh[h][:osz, :],
                        lhsT=wb8_sb[:, g, :, osl],
                        rhs=xq8_sb[:, g, :, h * 512:(h + 1) * 512],
                        start=(g == 0), stop=False,
                        perf_mode=mybir.MatmulPerfMode.DoubleRow,
                    )
                else:
                    for i in range(2):
                        ps_i = (g - NG8) * 2 + i
                        nc.tensor.matmul(
                            psts_jh[h][:osz, :],
                            lhsT=whf_sb[:, ps_i, osl],
                            rhs=xhf_sb[:, ps_i, h * 512:(h + 1) * 512],
                            start=False, stop=(g == NG - 1 and i == 1),
                        )
            if h == 0:
                ot = opool.tile([128, 512], F32, name="ot", tag="ot")
                nc.scalar.activation(
                    ot[:osz, :], psts_jh[h][:osz, :],
                    mybir.ActivationFunctionType.Identity,
                )
                nc.sync.dma_start(
                    out=out_d[j * 128:j * 128 + osz, 0:512],
                    in_=ot[:osz, :],
                )
            else:
                # final eviction on DVE (idle by now; ACT took half 0)
                ot = opool.tile([128, 512], F32, name="ot", tag="ot")
                nc.vector.tensor_scalar(
                    out=ot[:osz, :], in0=psts_jh[h][:osz, :],
                    scalar1=0.0, scalar2=None,
                    op0=mybir.AluOpType.add,
                )
                nc.scalar.dma_start(
                    out=out_d[j * 128:j * 128 + osz, 512:1024],
                    in_=ot[:osz, :],
                )

    _legalize_waits(nc)
    return nc


def _prep_inputs(x: np.ndarray, bp: np.ndarray):
    x = np.ascontiguousarray(x, dtype=np.float32)
    # x.T is [k, t]; k = jt*1024 + q*8 + p -> [jt, q, p, t]
    xr = np.ascontiguousarray(x.T).reshape(4, 128, 8, TOKENS)
    # fp8 groups (jt 0,1): [q, (jt,pp), i, t]
    a = xr[:2].reshape(2, 128, 4, 2, TOKENS)          # [jt, q, pp, i, t]
    xq8 = np.ascontiguousarray(
        a.transpose(1, 0, 2, 3, 4).reshape(128, 2 * NG8 * TOKENS)
    ).astype(ml_dtypes.float8_e4m3fn)
    # fp16 groups (jt 2,3): [q, (jt-2, p), t]
    xhf = np.ascontiguousarray(
        xr[2:].transpose(1, 0, 2, 3).reshape(128, NPS * TOKENS)
    ).astype(np.float16)

    bytes_m = bp.reshape(OUT_F, IN_F // 8).astype(np.uint8)   # [o, B]
    bph_full = np.ascontiguousarray(
        bytes_m.T.reshape(4, 128, OUT_F).transpose(1, 0, 2)
    )  # [q, jt, o]

    in_maps = []
    for c in range(N_CORES):
        sl = slice(c * OS, (c + 1) * OS)
        bph_c = np.ascontiguousarray(bph_full[:, :, sl]).reshape(128, 4 * OS)
        in_maps.append({
            "xq8": xq8,
            "xhf": xhf,
            "bph": bph_c,
            "bph16": bph_c.astype(np.uint16),
        })
    return in_maps


def _run(x: np.ndarray, bp: np.ndarray, **spmd_kwargs):
    if "nc" not in _CACHE:
        _CACHE["nc"] = _build_module()
    nc = _CACHE["nc"]
    in_maps = _prep_inputs(x, bp)
    res = run_bass_kernel_spmd(
        nc, in_maps, core_ids=list(range(N_CORES)), **spmd_kwargs
    )
    # per-core out is [OS, TOKENS]; gather + transpose to [TOKENS, OUT_F]
    out = np.concatenate([r["out"] for r in res.results], axis=0)
    out = np.ascontiguousarray(out.T)
    return out, res


def _host_reference(x: np.ndarray, bp: np.ndarray) -> np.ndarray:
    # Safety net for inputs outside the fast path's envelope.
    shifts = np.arange(7, -1, -1)
    bits = (bp.astype(np.int64)[:, None] >> shifts) & 1
    w = bits.reshape(OUT_F, IN_F).astype(np.float32) * 2 - 1
    return (x @ w.T).astype(np.float32)


def kernel(x: np.ndarray, bp: np.ndarray) -> np.ndarray:
    x = np.asarray(x, dtype=np.float32)
    bp = np.asarray(bp)
    # e4m3 saturates above 240 and fp16 above 65504; stay well inside.
    if (not np.isfinite(x).all()) or np.abs(x).max() >= 200.0 \
            or bp.min() < 0 or bp.max() > 255:
        return _host_reference(x, bp)
    out, _ = _run(x, bp)
    return out
